# revision 23
# baseline (speedup 1.0000x reference)
"""Trainium2 Bass kernel for BondValencePredictor (sparse_attention).

Reference computation (per batch sample a of B=64, A=128 atoms, C=512 in-feats):
    keys    = leaky_relu(x @ Wk + bk, 0.1)                  # [B, A, 256]
    queries = leaky_relu(x @ Wq + bq, 0.1)                  # [B, A, 6144]
              .reshape(B, A, 256, 4, 6)
    bdata[a,b,d,e,f] = sum_c keys[a,b,c] * queries[a,f,c,d,e]
    out = where(f > b, -inf, bdata)                         # [B, A, 4, 6, A]

Sharding: data-parallel over batch — 8 NeuronCores x 8 samples each; weights
replicated, no collectives.

Per-core layout strategy (all matmuls in float32r = full-rate fp32):
  - x is fed transposed: xT [512, 1024] (tokens = 8 samples x 128 atoms), so
    both projections produce channel-major outputs directly (channel on the
    PSUM partition dim, tokens on the free dim, N=512 moving operand).
  - Wq columns are host-permuted from c*24+de to de*256+c so each de-group's
    256 hid-channels are contiguous -> the einsum's rhs slices need no
    on-chip transpose: bdata[b, de, f] = sum_c keysT[c, b] * qT_de[c, f].
  - de (the 4x6 bond-type/valence grid) is processed in blocks of
    [2,4,4,4,4,4,2]; each einsum matmul covers one (sample, c-chunk) against
    nde x 128 atoms (moving dim >= 256 keeps f32r full rate), accumulating
    over the two 128-wide c-chunks in PSUM.
  - The strict upper-triangular mask is applied by adding a 0/-inf tile.
  - Projection blocks are emitted one block ahead of einsum blocks (einsum
    samples spread between projection de-groups) so the PE waits on neither
    the Prelu epilogue nor the DVE mask-add epilogue; dummy warm-up matmuls
    at the start keep the PE clock un-throttled through the DMA fill.
"""

import numpy as np

import concourse.bass as bass
import concourse.mybir as mybir
from concourse.tile import TileContext
from concourse.bass_utils import run_bass_kernel_spmd

F32 = mybir.dt.float32
F32R = mybir.dt.float32r
AF = mybir.ActivationFunctionType

B, A, CIN = 64, 128, 512
HID = 256
DE = 24                  # 4 bond types x 6 valences
QF = HID * DE            # 6144
NCORES = 8
BPC = B // NCORES        # samples per core
NTOK = BPC * A           # tokens per core
LEAK = 0.1
DBLK = 4                 # de's per block
NBLK = DE // DBLK
KCH = CIN // 128         # contraction chunks


def _split_excess_waits(nc, max_waits=1):
    """Walrus codegen allows only one sem wait per instruction; Tile's
    kernel-tail drain aggregates one wait per logical proc. Hoist excess
    waits onto same-engine drains inserted immediately before (engines
    execute their stream in order, so the happens-before is preserved)."""
    for f in nc.m.functions:
        for bb in f.blocks:
            insts = bb.instructions
            i = 0
            while i < len(insts):
                ins = insts[i]
                si = ins.sync_info
                if si is not None and si.on_wait and len(si.on_wait) > max_waits:
                    waits = list(si.on_wait)
                    extra, keep = waits[:-max_waits], waits[-max_waits:]
                    new_insts = []
                    k = 0
                    while extra:
                        chunk, extra = extra[:max_waits], extra[max_waits:]
                        nd = mybir.InstNoOp(
                            name=f"{ins.name}-sw{k}", ins=[], outs=[])
                        nd.engine = ins.engine
                        nd.sync_info = mybir.SyncInfo(on_wait=chunk, on_update=[])
                        new_insts.append(nd)
                        k += 1
                    ins.sync_info = mybir.SyncInfo(
                        on_wait=keep, on_update=list(si.on_update or []))
                    insts[i:i] = new_insts
                    i += len(new_insts)
                i += 1


def _r(ap):
    return ap.bitcast(F32R)


def _build(split_waits=True):
    nc = bass.Bass()
    xt_d = nc.dram_tensor("xt", [CIN, NTOK], F32R, kind="ExternalInput")
    wk_d = nc.dram_tensor("wk", [CIN, HID], F32R, kind="ExternalInput")
    bk_d = nc.dram_tensor("bk2", [128, 2], F32, kind="ExternalInput")
    wq_d = nc.dram_tensor("wq", [CIN, QF], F32R, kind="ExternalInput")
    bq_d = nc.dram_tensor("bq48", [128, 48], F32, kind="ExternalInput")
    mask_d = nc.dram_tensor("mask4", [128, DBLK * A], F32, kind="ExternalInput")
    out_d = nc.dram_tensor("out", [BPC, A, DE, A], F32, kind="ExternalOutput")

    # variable de-blocks: a half-size first block halves the weight bytes on
    # the startup critical path; a half-size last block halves the trailing
    # einsum epilogue. einsum moving dim = nde*128 >= 256 keeps f32r full rate.
    BLK_N = [2, 4, 4, 4, 4, 4, 2]
    BLK_0 = [sum(BLK_N[:i]) for i in range(len(BLK_N))]
    NB = len(BLK_N)

    with TileContext(nc) as tc:
        with (
            tc.tile_pool(name="const", bufs=1) as cpool,
            tc.tile_pool(name="wqp", bufs=2) as wqpool,
            tc.tile_pool(name="qtp", bufs=2) as qtpool,
            tc.tile_pool(name="obp", bufs=6) as opool,
            tc.tile_pool(name="psp", bufs=4, space="PSUM") as ps_p,
            tc.tile_pool(name="pse", bufs=4, space="PSUM") as ps_e,
        ):
            # ---- PE warm-up: dummy matmuls with no DMA dependency keep the
            # HAM activity window busy while inputs stream in ----
            scratch = cpool.tile([128, 512], mybir.dt.bfloat16)
            nc.vector.memset(scratch[:], 0.0)
            ps_w = ps_e.tile([128, 512], F32, name="ps_warm", tag="pe")
            for _ in range(12):
                nc.tensor.matmul(ps_w[:], scratch[:, 0:128], scratch[:],
                                 start=True, stop=True)

            # ---- resident inputs; DMA issue order == consumption order ----
            wk = cpool.tile([128, KCH, HID], F32R)
            nc.sync.dma_start(wk[:], wk_d[:, :].rearrange("(k p) m -> p k m", p=128))
            bk2 = cpool.tile([128, 2], F32)
            nc.sync.dma_start(bk2[:], bk_d[:, :])
            bq48 = cpool.tile([128, 48], F32)
            nc.sync.dma_start(bq48[:], bq_d[:, :])
            NTT = NTOK // 512
            xt_t = [[None] * NTT for _ in range(KCH)]
            for k in range(KCH):
                xt_t[k][0] = cpool.tile([128, 512], F32R, name=f"xt_{k}_0",
                                        tag=f"xt_{k}_0")
                nc.sync.dma_start(
                    xt_t[k][0][:], xt_d[k * 128:(k + 1) * 128, 0:512])

            wq_tiles = {}

            def wq_dma(bi):
                """Weight DMAs for one block (>=2KB descriptor rows — smaller
                slices tank aggregate HBM throughput)."""
                d0, nde = BLK_0[bi], BLK_N[bi]
                wq_t = []
                for k in range(KCH):
                    w = wqpool.tile([128, nde * HID], F32R,
                                    name=f"wq_{bi}_{k}", tag=f"wq_{k}")
                    nc.sync.dma_start(
                        w[:], wq_d[k * 128:(k + 1) * 128,
                                   d0 * HID:(d0 + nde) * HID])
                    wq_t.append(w)
                wq_tiles[bi] = wq_t

            # block-0 weights interleaved with the tt=1 x chunks: arrival
            # order matches consumption (keys-tt1 matmuls, then block 0)
            wq0_t = []
            for k in range(KCH):
                w = wqpool.tile([128, BLK_N[0] * HID], F32R, name=f"wq_0_{k}",
                                tag=f"wq_{k}")
                nc.sync.dma_start(w[:], wq_d[k * 128:(k + 1) * 128,
                                             0:BLK_N[0] * HID])
                wq0_t.append(w)
                xt_t[k][1] = cpool.tile([128, 512], F32R, name=f"xt_{k}_1",
                                        tag=f"xt_{k}_1")
                nc.sync.dma_start(
                    xt_t[k][1][:], xt_d[k * 128:(k + 1) * 128, 512:1024])
            wq_tiles[0] = wq0_t

            # ---- keys projection: keysT[c-chunk][c, tok] ----
            keysT = cpool.tile([128, 2, NTOK], F32R)
            for tt in range(NTT):
                for hh in range(2):
                    ps = ps_p.tile([128, 512], F32, name=f"psk_{tt}_{hh}",
                                   tag="ps")
                    for k in range(KCH):
                        nc.tensor.matmul(
                            ps[:],
                            wk[:, k, hh * 128:(hh + 1) * 128],
                            xt_t[k][tt][:],
                            start=(k == 0), stop=(k == KCH - 1),
                        )
                    nc.scalar.activation(
                        keysT[:, hh, tt * 512:(tt + 1) * 512], ps[:],
                        AF.Prelu, bias=bk2[:, hh:hh + 1], scale=1.0, alpha=LEAK)
                # bridge: keep the PE (and HAM) busy while the tt=1 x
                # chunks and the first wq slices stream in
                for _ in range(10 if tt == 0 else 6):
                    nc.tensor.matmul(ps_w[:], scratch[:, 0:128],
                                     scratch[:], start=True, stop=True)

            qt_tiles = {}

            def proj_de(bi, de_i):
                """Query projection for one de (16 matmuls, 2 activations)."""
                d0, nde = BLK_0[bi], BLK_N[bi]
                de = d0 + de_i
                if de_i == 0:
                    qt_tiles[bi] = qtpool.tile([128, 2, nde, NTOK], F32R,
                                               name=f"qt_{bi}", tag="qt")
                qt = qt_tiles[bi]
                wq_t = wq_tiles[bi]
                for cc in range(2):
                    pss = [ps_p.tile([128, 512], F32,
                                     name=f"psq_{de}_{cc}_{t}", tag="ps")
                           for t in range(NTT)]
                    for k in range(KCH):
                        for tt in range(NTT):
                            nc.tensor.matmul(
                                pss[tt][:],
                                wq_t[k][:, de_i * HID + cc * 128:
                                        de_i * HID + (cc + 1) * 128],
                                xt_t[k][tt][:],
                                start=(k == 0), stop=(k == KCH - 1),
                            )
                    j = de * 2 + cc
                    for tt in range(NTT):
                        nc.scalar.activation(
                            qt[:, cc, de_i, tt * 512:(tt + 1) * 512],
                            pss[tt][:],
                            AF.Prelu, bias=bq48[:, j:j + 1], scale=1.0,
                            alpha=LEAK)

            def einsum_a(bi, a):
                """bdata for one (sample, de-block): 2 matmuls + mask + store."""
                d0, nde = BLK_0[bi], BLK_N[bi]
                qt = qt_tiles[bi]
                pe = ps_e.tile([128, DBLK * A], F32, name=f"pe_{bi}_{a}",
                               tag="pe")
                for cc in range(2):
                    nc.tensor.matmul(
                        pe[:, 0:nde * A],
                        keysT[:, cc, a * A:(a + 1) * A],
                        qt[:, cc, :, a * A:(a + 1) * A],
                        start=(cc == 0), stop=(cc == 1),
                    )
                ob = opool.tile([128, DBLK * A], F32, name=f"ob_{bi}_{a}",
                                tag="ob")
                nc.vector.tensor_add(ob[:, 0:nde * A], pe[:, 0:nde * A],
                                     mask4[:, 0:nde * A])
                # final block: issue stores from the (by then idle) scalar
                # engine so the tail DMA issues don't serialize on sync
                dma_eng = nc.scalar if bi == NB - 1 else nc.sync
                dma_eng.dma_start(
                    out_d[a, :, d0:d0 + nde, :],
                    ob[:, 0:nde * A].rearrange("p (q m) -> p q m", m=A))

            # mask tile is only needed by the first einsum; keep it out of
            # the critical early DMA stream
            mask4 = cpool.tile([128, DBLK * A], F32)

            # one-block software pipeline: proj(bi+1) is emitted before
            # einsum(bi), weight DMAs before the previous block's out-DMAs,
            # einsum samples spread between projection groups so the DVE
            # mask-add epilogue never backpressures the PE.
            for de_i in range(BLK_N[0]):
                proj_de(0, de_i)
                if de_i == 0:
                    nc.sync.dma_start(mask4[:], mask_d[:, :])
            for bi in range(NB):
                nxt = bi + 1
                if nxt < NB:
                    wq_dma(nxt)
                    nde = BLK_N[nxt]
                    for de_i in range(nde):
                        proj_de(nxt, de_i)
                        a0 = de_i * BPC // nde
                        a1 = (de_i + 1) * BPC // nde
                        for a in range(a0, a1):
                            einsum_a(bi, a)
                else:
                    for a in range(BPC):
                        einsum_a(bi, a)

    if split_waits:
        _split_excess_waits(nc)
    return nc


_NC = None
LAST_RESULTS = None  # BassKernelResults of the most recent kernel() call


def kernel(x, Wk, bk, Wq, bq, _trace=False):
    global _NC, LAST_RESULTS
    if _NC is None:
        _NC = _build()

    x = np.asarray(x, np.float32)
    Wk = np.ascontiguousarray(np.asarray(Wk, np.float32))
    bk = np.asarray(bk, np.float32)
    Wq = np.asarray(Wq, np.float32)
    bq = np.asarray(bq, np.float32)

    # Wq columns c*24+de -> de*256+c; bias into [128, de*2+cc] per-partition form
    wq_perm = np.ascontiguousarray(
        Wq.reshape(CIN, HID, DE).transpose(0, 2, 1).reshape(CIN, QF))
    bq48 = np.ascontiguousarray(
        bq.reshape(2, 128, DE).transpose(1, 2, 0).reshape(128, DE * 2))
    bk2 = np.ascontiguousarray(bk.reshape(2, 128).T)
    m = np.where(np.arange(A)[None, :] > np.arange(A)[:, None],
                 -np.inf, 0.0).astype(np.float32)
    mask4 = np.ascontiguousarray(np.tile(m, (1, DBLK)))

    in_maps = []
    for c in range(NCORES):
        xs = x[c * BPC:(c + 1) * BPC].reshape(NTOK, CIN)
        in_maps.append({
            "xt": np.ascontiguousarray(xs.T),
            "wk": Wk,
            "bk2": bk2,
            "wq": wq_perm,
            "bq48": bq48,
            "mask4": mask4,
        })

    res = run_bass_kernel_spmd(_NC, in_maps, core_ids=list(range(NCORES)),
                               trace=_trace)
    LAST_RESULTS = res
    out = np.concatenate([res.results[c]["out"] for c in range(NCORES)], axis=0)
    return np.ascontiguousarray(
        out.reshape(B, A, 4, 6, A)).astype(np.float32, copy=False)


# revision 24
# speedup vs baseline: 1.0072x; 1.0072x over previous
"""Trainium2 Bass kernel for BondValencePredictor (sparse_attention).

Reference computation (per batch sample a of B=64, A=128 atoms, C=512 in-feats):
    keys    = leaky_relu(x @ Wk + bk, 0.1)                  # [B, A, 256]
    queries = leaky_relu(x @ Wq + bq, 0.1)                  # [B, A, 6144]
              .reshape(B, A, 256, 4, 6)
    bdata[a,b,d,e,f] = sum_c keys[a,b,c] * queries[a,f,c,d,e]
    out = where(f > b, -inf, bdata)                         # [B, A, 4, 6, A]

Sharding: data-parallel over batch — 8 NeuronCores x 8 samples each; weights
replicated, no collectives.

Per-core layout strategy (all matmuls in float32r = full-rate fp32):
  - x is fed transposed: xT [512, 1024] (tokens = 8 samples x 128 atoms), so
    both projections produce channel-major outputs directly (channel on the
    PSUM partition dim, tokens on the free dim, N=512 moving operand).
  - Wq columns are host-permuted from c*24+de to de*256+c so each de-group's
    256 hid-channels are contiguous -> the einsum's rhs slices need no
    on-chip transpose: bdata[b, de, f] = sum_c keysT[c, b] * qT_de[c, f].
  - de (the 4x6 bond-type/valence grid) is processed in blocks of
    [2,4,4,4,4,4,2]; each einsum matmul covers one (sample, c-chunk) against
    nde x 128 atoms (moving dim >= 256 keeps f32r full rate), accumulating
    over the two 128-wide c-chunks in PSUM.
  - The strict upper-triangular mask is applied by adding a 0/-inf tile.
  - Projection blocks are emitted one block ahead of einsum blocks (einsum
    samples spread between projection de-groups) so the PE waits on neither
    the Prelu epilogue nor the DVE mask-add epilogue; dummy warm-up matmuls
    at the start keep the PE clock un-throttled through the DMA fill.
"""

import numpy as np

import concourse.bass as bass
import concourse.mybir as mybir
from concourse.tile import TileContext
from concourse.bass_utils import run_bass_kernel_spmd

F32 = mybir.dt.float32
F32R = mybir.dt.float32r
AF = mybir.ActivationFunctionType

B, A, CIN = 64, 128, 512
HID = 256
DE = 24                  # 4 bond types x 6 valences
QF = HID * DE            # 6144
NCORES = 8
BPC = B // NCORES        # samples per core
NTOK = BPC * A           # tokens per core
LEAK = 0.1
DBLK = 4                 # de's per block
NBLK = DE // DBLK
KCH = CIN // 128         # contraction chunks


def _split_excess_waits(nc, max_waits=1):
    """Walrus codegen allows only one sem wait per instruction; Tile's
    kernel-tail drain aggregates one wait per logical proc. Hoist excess
    waits onto same-engine drains inserted immediately before (engines
    execute their stream in order, so the happens-before is preserved)."""
    for f in nc.m.functions:
        for bb in f.blocks:
            insts = bb.instructions
            i = 0
            while i < len(insts):
                ins = insts[i]
                si = ins.sync_info
                if si is not None and si.on_wait and len(si.on_wait) > max_waits:
                    waits = list(si.on_wait)
                    extra, keep = waits[:-max_waits], waits[-max_waits:]
                    new_insts = []
                    k = 0
                    while extra:
                        chunk, extra = extra[:max_waits], extra[max_waits:]
                        nd = mybir.InstNoOp(
                            name=f"{ins.name}-sw{k}", ins=[], outs=[])
                        nd.engine = ins.engine
                        nd.sync_info = mybir.SyncInfo(on_wait=chunk, on_update=[])
                        new_insts.append(nd)
                        k += 1
                    ins.sync_info = mybir.SyncInfo(
                        on_wait=keep, on_update=list(si.on_update or []))
                    insts[i:i] = new_insts
                    i += len(new_insts)
                i += 1


def _r(ap):
    return ap.bitcast(F32R)


def _build(split_waits=True):
    nc = bass.Bass()
    xt_d = nc.dram_tensor("xt", [CIN, NTOK], F32R, kind="ExternalInput")
    wk_d = nc.dram_tensor("wk", [CIN, HID], F32R, kind="ExternalInput")
    bk_d = nc.dram_tensor("bk2", [128, 2], F32, kind="ExternalInput")
    wq_d = nc.dram_tensor("wq", [CIN, QF], F32R, kind="ExternalInput")
    bq_d = nc.dram_tensor("bq48", [128, 48], F32, kind="ExternalInput")
    mask_d = nc.dram_tensor("mask4", [128, DBLK * A], F32, kind="ExternalInput")
    out_d = nc.dram_tensor("out", [BPC, A, DE, A], F32, kind="ExternalOutput")

    # variable de-blocks: a half-size first block halves the weight bytes on
    # the startup critical path; a half-size last block halves the trailing
    # einsum epilogue. einsum moving dim = nde*128 >= 256 keeps f32r full rate.
    BLK_N = [2, 4, 4, 4, 4, 4, 2]
    BLK_0 = [sum(BLK_N[:i]) for i in range(len(BLK_N))]
    NB = len(BLK_N)

    with TileContext(nc) as tc:
        with (
            tc.tile_pool(name="const", bufs=1) as cpool,
            tc.tile_pool(name="wqp", bufs=2) as wqpool,
            tc.tile_pool(name="qtp", bufs=2) as qtpool,
            tc.tile_pool(name="obp", bufs=6) as opool,
            tc.tile_pool(name="psp", bufs=4, space="PSUM") as ps_p,
            tc.tile_pool(name="pse", bufs=4, space="PSUM") as ps_e,
        ):
            # ---- PE warm-up: dummy matmuls with no DMA dependency keep the
            # HAM activity window busy while inputs stream in ----
            scratch = cpool.tile([128, 512], mybir.dt.bfloat16)
            # gpsimd (idle at start) clears the scratch ~1us before DVE could,
            # so the HAM warm-up matmuls begin that much earlier
            nc.gpsimd.memset(scratch[:], 0.0)
            ps_w = ps_e.tile([128, 512], F32, name="ps_warm", tag="pe")
            for _ in range(12):
                nc.tensor.matmul(ps_w[:], scratch[:, 0:128], scratch[:],
                                 start=True, stop=True)

            # ---- resident inputs; DMA issue order == consumption order ----
            wk = cpool.tile([128, KCH, HID], F32R)
            nc.sync.dma_start(wk[:], wk_d[:, :].rearrange("(k p) m -> p k m", p=128))
            bk2 = cpool.tile([128, 2], F32)
            nc.sync.dma_start(bk2[:], bk_d[:, :])
            bq48 = cpool.tile([128, 48], F32)
            nc.sync.dma_start(bq48[:], bq_d[:, :])
            NTT = NTOK // 512
            xt_t = [[None] * NTT for _ in range(KCH)]
            for k in range(KCH):
                xt_t[k][0] = cpool.tile([128, 512], F32R, name=f"xt_{k}_0",
                                        tag=f"xt_{k}_0")
                nc.sync.dma_start(
                    xt_t[k][0][:], xt_d[k * 128:(k + 1) * 128, 0:512])

            wq_tiles = {}

            def wq_dma(bi):
                """Weight DMAs for one block (>=2KB descriptor rows — smaller
                slices tank aggregate HBM throughput)."""
                d0, nde = BLK_0[bi], BLK_N[bi]
                wq_t = []
                for k in range(KCH):
                    w = wqpool.tile([128, nde * HID], F32R,
                                    name=f"wq_{bi}_{k}", tag=f"wq_{k}")
                    nc.sync.dma_start(
                        w[:], wq_d[k * 128:(k + 1) * 128,
                                   d0 * HID:(d0 + nde) * HID])
                    wq_t.append(w)
                wq_tiles[bi] = wq_t

            # block-0 weights interleaved with the tt=1 x chunks: arrival
            # order matches consumption (keys-tt1 matmuls, then block 0)
            wq0_t = []
            for k in range(KCH):
                w = wqpool.tile([128, BLK_N[0] * HID], F32R, name=f"wq_0_{k}",
                                tag=f"wq_{k}")
                nc.sync.dma_start(w[:], wq_d[k * 128:(k + 1) * 128,
                                             0:BLK_N[0] * HID])
                wq0_t.append(w)
                xt_t[k][1] = cpool.tile([128, 512], F32R, name=f"xt_{k}_1",
                                        tag=f"xt_{k}_1")
                nc.sync.dma_start(
                    xt_t[k][1][:], xt_d[k * 128:(k + 1) * 128, 512:1024])
            wq_tiles[0] = wq0_t

            # ---- keys projection: keysT[c-chunk][c, tok] ----
            keysT = cpool.tile([128, 2, NTOK], F32R)
            for tt in range(NTT):
                for hh in range(2):
                    ps = ps_p.tile([128, 512], F32, name=f"psk_{tt}_{hh}",
                                   tag="ps")
                    for k in range(KCH):
                        nc.tensor.matmul(
                            ps[:],
                            wk[:, k, hh * 128:(hh + 1) * 128],
                            xt_t[k][tt][:],
                            start=(k == 0), stop=(k == KCH - 1),
                        )
                    nc.scalar.activation(
                        keysT[:, hh, tt * 512:(tt + 1) * 512], ps[:],
                        AF.Prelu, bias=bk2[:, hh:hh + 1], scale=1.0, alpha=LEAK)
                # bridge: keep the PE (and HAM) busy while the tt=1 x
                # chunks and the first wq slices stream in
                for _ in range(10 if tt == 0 else 6):
                    nc.tensor.matmul(ps_w[:], scratch[:, 0:128],
                                     scratch[:], start=True, stop=True)

            qt_tiles = {}

            def proj_de(bi, de_i):
                """Query projection for one de (16 matmuls, 2 activations)."""
                d0, nde = BLK_0[bi], BLK_N[bi]
                de = d0 + de_i
                if de_i == 0:
                    qt_tiles[bi] = qtpool.tile([128, 2, nde, NTOK], F32R,
                                               name=f"qt_{bi}", tag="qt")
                qt = qt_tiles[bi]
                wq_t = wq_tiles[bi]
                for cc in range(2):
                    pss = [ps_p.tile([128, 512], F32,
                                     name=f"psq_{de}_{cc}_{t}", tag="ps")
                           for t in range(NTT)]
                    for k in range(KCH):
                        for tt in range(NTT):
                            nc.tensor.matmul(
                                pss[tt][:],
                                wq_t[k][:, de_i * HID + cc * 128:
                                        de_i * HID + (cc + 1) * 128],
                                xt_t[k][tt][:],
                                start=(k == 0), stop=(k == KCH - 1),
                            )
                    j = de * 2 + cc
                    for tt in range(NTT):
                        nc.scalar.activation(
                            qt[:, cc, de_i, tt * 512:(tt + 1) * 512],
                            pss[tt][:],
                            AF.Prelu, bias=bq48[:, j:j + 1], scale=1.0,
                            alpha=LEAK)

            def einsum_a(bi, a):
                """bdata for one (sample, de-block): 2 matmuls + mask + store."""
                d0, nde = BLK_0[bi], BLK_N[bi]
                qt = qt_tiles[bi]
                pe = ps_e.tile([128, DBLK * A], F32, name=f"pe_{bi}_{a}",
                               tag="pe")
                for cc in range(2):
                    nc.tensor.matmul(
                        pe[:, 0:nde * A],
                        keysT[:, cc, a * A:(a + 1) * A],
                        qt[:, cc, :, a * A:(a + 1) * A],
                        start=(cc == 0), stop=(cc == 1),
                    )
                ob = opool.tile([128, DBLK * A], F32, name=f"ob_{bi}_{a}",
                                tag="ob")
                nc.vector.tensor_add(ob[:, 0:nde * A], pe[:, 0:nde * A],
                                     mask4[:, 0:nde * A])
                # final block: issue stores from the (by then idle) scalar
                # engine so the tail DMA issues don't serialize on sync
                dma_eng = nc.scalar if bi == NB - 1 else nc.sync
                dma_eng.dma_start(
                    out_d[a, :, d0:d0 + nde, :],
                    ob[:, 0:nde * A].rearrange("p (q m) -> p q m", m=A))

            # mask tile is only needed by the first einsum; keep it out of
            # the critical early DMA stream
            mask4 = cpool.tile([128, DBLK * A], F32)

            # one-block software pipeline: proj(bi+1) is emitted before
            # einsum(bi), weight DMAs before the previous block's out-DMAs,
            # einsum samples spread between projection groups so the DVE
            # mask-add epilogue never backpressures the PE.
            for de_i in range(BLK_N[0]):
                proj_de(0, de_i)
                if de_i == 0:
                    nc.sync.dma_start(mask4[:], mask_d[:, :])
            for bi in range(NB):
                nxt = bi + 1
                if nxt < NB:
                    wq_dma(nxt)
                    nde = BLK_N[nxt]
                    for de_i in range(nde):
                        proj_de(nxt, de_i)
                        a0 = de_i * BPC // nde
                        a1 = (de_i + 1) * BPC // nde
                        for a in range(a0, a1):
                            einsum_a(bi, a)
                else:
                    for a in range(BPC):
                        einsum_a(bi, a)

    if split_waits:
        _split_excess_waits(nc)
    return nc


_NC = None
LAST_RESULTS = None  # BassKernelResults of the most recent kernel() call


def kernel(x, Wk, bk, Wq, bq, _trace=False):
    global _NC, LAST_RESULTS
    if _NC is None:
        _NC = _build()

    x = np.asarray(x, np.float32)
    Wk = np.ascontiguousarray(np.asarray(Wk, np.float32))
    bk = np.asarray(bk, np.float32)
    Wq = np.asarray(Wq, np.float32)
    bq = np.asarray(bq, np.float32)

    # Wq columns c*24+de -> de*256+c; bias into [128, de*2+cc] per-partition form
    wq_perm = np.ascontiguousarray(
        Wq.reshape(CIN, HID, DE).transpose(0, 2, 1).reshape(CIN, QF))
    bq48 = np.ascontiguousarray(
        bq.reshape(2, 128, DE).transpose(1, 2, 0).reshape(128, DE * 2))
    bk2 = np.ascontiguousarray(bk.reshape(2, 128).T)
    m = np.where(np.arange(A)[None, :] > np.arange(A)[:, None],
                 -np.inf, 0.0).astype(np.float32)
    mask4 = np.ascontiguousarray(np.tile(m, (1, DBLK)))

    in_maps = []
    for c in range(NCORES):
        xs = x[c * BPC:(c + 1) * BPC].reshape(NTOK, CIN)
        in_maps.append({
            "xt": np.ascontiguousarray(xs.T),
            "wk": Wk,
            "bk2": bk2,
            "wq": wq_perm,
            "bq48": bq48,
            "mask4": mask4,
        })

    res = run_bass_kernel_spmd(_NC, in_maps, core_ids=list(range(NCORES)),
                               trace=_trace)
    LAST_RESULTS = res
    out = np.concatenate([res.results[c]["out"] for c in range(NCORES)], axis=0)
    return np.ascontiguousarray(
        out.reshape(B, A, 4, 6, A)).astype(np.float32, copy=False)
